# revision 1
# baseline (speedup 1.0000x reference)
"""DiT block kernel for Trainium2, data-parallel over batch across 8 NeuronCores.

Problem: nn_DiTBlock — B=8, S=1024, E=512, H=8 (head_dim = E = 512).
Sharding: batch element b -> core b. Each core runs the full DiT block on its
(S, E) slice with replicated weights; no collectives.

Per-core dataflow (activations kept transposed so the contraction dim sits on
partitions; float32r matmuls = full-rate PE with ~1.5e-4 relative rounding):
  AdaLN matvecs (PE) -> LN1 stats (DVE bn_stats) -> PE-transpose with fused
  modulate (ACT per-partition scale/bias) -> y^T
  -> software-pipelined head loop: QK(h) emitted between scores(h-1) and
     AV(h-1) so exp/tree latency hides under PE work
  -> residual -> LN2 (same transpose-modulate trick) -> FFN (relu in ACT)
  -> out = y + (h @ f2w + f2b) * alpha2.
"""
import sys
import numpy as np

sys.path.insert(0, '/opt/trn_rl_repo')

B, S, E, H = 8, 1024, 512, 8
HE = H * E          # 4096
FF = 4 * E          # 2048
EPS = 1e-5
SCALE = 1.0 / 32.0  # 1/sqrt(S)
N_CORES = 8

TRACE = False       # set by test harness to capture an NTFF profile
TRACE_DIR = None

_CACHE = {}


def _build():
    from contextlib import ExitStack
    import concourse.bass as bass
    import concourse.tile as tile
    from concourse import bacc, mybir
    f32 = mybir.dt.float32
    f32r = mybir.dt.float32r
    bf16 = mybir.dt.bfloat16
    AF = mybir.ActivationFunctionType
    ALU = mybir.AluOpType

    nc = bacc.Bacc("TRN2", target_bir_lowering=False, debug=False,
                   num_devices=N_CORES)

    # ---- DRAM parameters --------------------------------------------------
    x_d = nc.dram_tensor("x", [S, E], f32, kind="ExternalInput").ap()
    cond_d = nc.dram_tensor("cond", [E, 1], f32, kind="ExternalInput").ap()

    adaln_w = {}
    adaln_b = {}
    for nm in ["g1", "be1", "a1", "g2", "be2", "a2"]:
        adaln_w[nm] = nc.dram_tensor(f"{nm}w", [E, E], f32,
                                     kind="ExternalInput").ap()
        adaln_b[nm] = nc.dram_tensor(f"{nm}b", [1, E], f32,
                                     kind="ExternalInput").ap()
    ln1g_d = nc.dram_tensor("ln1g", [1, E], f32, kind="ExternalInput").ap()
    ln1b_d = nc.dram_tensor("ln1b", [1, E], f32, kind="ExternalInput").ap()
    ln2g_d = nc.dram_tensor("ln2g", [1, E], f32, kind="ExternalInput").ap()
    ln2b_d = nc.dram_tensor("ln2b", [1, E], f32, kind="ExternalInput").ap()
    wq_d = nc.dram_tensor("wq", [E, HE], f32r, kind="ExternalInput").ap()
    wk_d = nc.dram_tensor("wk", [E, HE], f32r, kind="ExternalInput").ap()
    wv_d = nc.dram_tensor("wv", [E, HE], f32r, kind="ExternalInput").ap()
    bq_d = nc.dram_tensor("bq", [1, HE], f32, kind="ExternalInput").ap()
    bk_d = nc.dram_tensor("bk", [1, HE], f32, kind="ExternalInput").ap()
    bv_d = nc.dram_tensor("bv", [1, HE], f32, kind="ExternalInput").ap()
    lvw_d = nc.dram_tensor("lvw", [HE, E], f32r, kind="ExternalInput").ap()
    lvb_d = nc.dram_tensor("lvb", [1, E], f32, kind="ExternalInput").ap()
    f1w_d = nc.dram_tensor("f1w", [E, FF], f32r, kind="ExternalInput").ap()
    f1b_d = nc.dram_tensor("f1b", [1, FF], f32, kind="ExternalInput").ap()
    f2w_d = nc.dram_tensor("f2w", [FF, E], f32r, kind="ExternalInput").ap()
    f2b_d = nc.dram_tensor("f2b", [1, E], f32, kind="ExternalInput").ap()
    ident_d = nc.dram_tensor("ident", [128, 128], f32r,
                             kind="ExternalInput").ap()
    out_d = nc.dram_tensor("out", [S, E], f32, kind="ExternalOutput").ap()

    with tile.TileContext(nc) as tc, ExitStack() as ctx:
        const = ctx.enter_context(tc.tile_pool(name="const", bufs=1))
        work = ctx.enter_context(tc.tile_pool(name="work", bufs=3))
        psum_mm = ctx.enter_context(
            tc.tile_pool(name="psum_mm", bufs=5, space="PSUM"))
        psum_tp = ctx.enter_context(
            tc.tile_pool(name="psum_tp", bufs=2, space="PSUM"))
        psum_row = ctx.enter_context(
            tc.tile_pool(name="psum_row", bufs=1, space="PSUM"))

        # ---- constants ----
        ident = const.tile([128, 128], f32r)
        nc.sync.dma_start(ident, ident_d)
        eps_t = const.tile([128, 1], f32)
        nc.vector.memset(eps_t, EPS)
        ones_col = const.tile([128, 1], f32)
        nc.vector.memset(ones_col, 1.0)

        cond_cols = const.tile([128, 4], f32)
        nc.sync.dma_start(cond_cols, cond_d.rearrange("(c p) o -> p (c o)", p=128))

        adp = ctx.enter_context(tc.tile_pool(name="adp", bufs=2))

        def adaln_cols(nm):
            """(cond @ W + b) laid out as [128, 4] e-columns (PE matvec)."""
            pcol = psum_tp.tile([128, 4], f32, name=f"pcol_{nm}", tag="ptp")
            for kc in range(4):
                adw = adp.tile([128, E], f32, name=f"adw_{nm}_{kc}", tag="adw")
                nc.sync.dma_start(adw, adaln_w[nm][kc * 128:(kc + 1) * 128, :])
                for ec in range(4):
                    nc.tensor.matmul(
                        pcol[:, ec:ec + 1],
                        adw[:, ec * 128:(ec + 1) * 128],
                        cond_cols[:, kc:kc + 1],
                        start=(kc == 0 and ec == 0),
                        stop=(kc == 3 and ec == 3))
            bcol = adp.tile([128, 4], f32, name=f"bcol_{nm}", tag="bcol")
            nc.sync.dma_start(
                bcol, adaln_b[nm].rearrange("o (c p) -> (o p) c", p=128))
            mcol = const.tile([128, 4], f32, name=f"mcol_{nm}")
            nc.vector.tensor_add(mcol, pcol, bcol)
            return mcol

        def adaln_rep(nm):
            """(cond @ W + b) replicated to [128, E] (row matvec + bcast)."""
            prow = psum_row.tile([1, E], f32, name=f"prow_{nm}", tag="prow")
            for kc in range(4):
                adw = adp.tile([128, E], f32, name=f"adw_{nm}_{kc}", tag="adw")
                nc.sync.dma_start(adw, adaln_w[nm][kc * 128:(kc + 1) * 128, :])
                nc.tensor.matmul(prow, cond_cols[:, kc:kc + 1], adw,
                                 start=(kc == 0), stop=(kc == 3))
            brow = adp.tile([1, E], f32, name=f"brow_{nm}", tag="brow")
            nc.sync.dma_start(brow, adaln_b[nm])
            arow = adp.tile([1, E], f32, name=f"arow_{nm}", tag="arow")
            nc.vector.tensor_add(arow, prow, brow)
            arep = const.tile([128, E], f32, name=f"arep_{nm}")
            nc.gpsimd.partition_broadcast(arep, arow)
            return arep

        def scale_shift(li, gcol, bcol, lng_d, lnb_d):
            """sc = ln_g*(1+gamma), bi = ln_b*(1+gamma)+beta, as [128,4] cols."""
            gp = const.tile([128, 4], f32, name=f"gp_{li}")
            nc.vector.tensor_scalar_add(gp, gcol, 1.0)
            lgc = adp.tile([128, 4], f32, name=f"lgc_{li}", tag="lgc")
            nc.sync.dma_start(lgc, lng_d.rearrange("o (c p) -> (o p) c", p=128))
            lbc = adp.tile([128, 4], f32, name=f"lbc_{li}", tag="lbc")
            nc.sync.dma_start(lbc, lnb_d.rearrange("o (c p) -> (o p) c", p=128))
            sc = const.tile([128, 4], f32, name=f"sc_{li}")
            nc.vector.tensor_mul(sc, lgc, gp)
            bi = const.tile([128, 4], f32, name=f"bi_{li}")
            nc.vector.tensor_mul(bi, lbc, gp)
            nc.vector.tensor_add(bi, bi, bcol)
            return sc, bi

        # Only g1/be1 gate the LN1->y^T critical path; defer the rest.
        sc1, bi1 = scale_shift(0, adaln_cols("g1"), adaln_cols("be1"),
                               ln1g_d, ln1b_d)

        # ---- persistent activation tiles ----
        yT = [const.tile([128, S], f32r, name=f"yT{c}") for c in range(4)]
        yT16 = [const.tile([128, S], bf16, name=f"yT16_{c}") for c in range(4)]
        y2acc = [const.tile([128, E], f32, name=f"y2acc{t}") for t in range(8)]

        def ln_stats(x_t, tagp):
            st = work.tile([128, 6], f32, name=f"st_{tagp}", tag=f"st_{tagp}")
            nc.vector.bn_stats(st, x_t)
            mv = work.tile([128, 2], f32, name=f"mv_{tagp}", tag=f"mv_{tagp}")
            nc.vector.bn_aggr(mv, st)
            rs = work.tile([128, 1], f32, name=f"rs_{tagp}", tag=f"rs_{tagp}")
            nc.scalar.activation(rs, mv[:, 1:2], AF.Sqrt, bias=eps_t, scale=1.0)
            nc.vector.reciprocal(rs, rs)
            xn = work.tile([128, E], f32r, name=f"xn_{tagp}", tag="wxn")
            nc.vector.tensor_scalar(xn, x_t, scalar1=mv[:, 0:1], scalar2=rs,
                                    op0=ALU.subtract, op1=ALU.mult)
            return xn

        def layernorm_transpose(src_tiles, scol, bcol, dst_T, tagp,
                                cast16=None):
            """LN over free dim of [128, E] tiles; PE-transpose 128-blocks in
            pairs sharing one PSUM bank; one fused modulate ACT per pair.
            (Only the pair's first transpose sets start=True: start clears the
            whole bank.)"""
            for t0 in range(0, 8, 2):
                xns = []
                for t in (t0, t0 + 1):
                    if src_tiles is None:
                        x_t = work.tile([128, E], f32, name=f"xt_{tagp}",
                                        tag="wbig")
                        nc.sync.dma_start(x_t, x_d[t * 128:(t + 1) * 128, :])
                    else:
                        x_t = src_tiles[t]
                    xns.append(ln_stats(x_t, tagp))
                for ec in range(4):
                    tp = psum_tp.tile([128, 256], f32r, name=f"tp_{tagp}",
                                      tag="ptp")
                    nc.tensor.matmul(tp[:, 0:128],
                                     xns[0][:, ec * 128:(ec + 1) * 128], ident,
                                     is_transpose=True, start=True, stop=False)
                    nc.tensor.matmul(tp[:, 128:256],
                                     xns[1][:, ec * 128:(ec + 1) * 128], ident,
                                     is_transpose=True, start=False, stop=True)
                    nc.scalar.activation(
                        dst_T[ec][:, t0 * 128:(t0 + 2) * 128], tp, AF.Identity,
                        bias=bcol[:, ec:ec + 1], scale=scol[:, ec:ec + 1])
                if cast16 is not None:
                    for c in range(4):
                        nc.vector.tensor_copy(
                            cast16[c][:, t0 * 128:(t0 + 2) * 128],
                            dst_T[c][:, t0 * 128:(t0 + 2) * 128])

        # f1w prefetched in quarter-chunks during heads 4-7 (small DMAs so
        # the sync queue never head-of-line blocks the per-head weights).
        ffp = ctx.enter_context(tc.tile_pool(name="ffp", bufs=1))
        f1w_t = [ffp.tile([128, FF], f32r, name=f"f1w{kc}", tag=f"f1w{kc}")
                 for kc in range(4)]
        f1bc = ffp.tile([128, 16], f32, tag="f1bc")

        def emit_f1w_quarter(q):
            for kc in range(4):
                nc.sync.dma_start(
                    f1w_t[kc][:, q * 512:(q + 1) * 512],
                    f1w_d[kc * 128:(kc + 1) * 128, q * 512:(q + 1) * 512])
            if q == 3:
                nc.sync.dma_start(
                    f1bc, f1b_d.rearrange("o (c p) -> (o p) c", p=128))

        hp_ctx = ExitStack()
        hp = hp_ctx.enter_context(tc.tile_pool(name="hp", bufs=1))

        def emit_qk_dma(h):
            hof = h * E
            wq_t = [hp.tile([128, E], bf16, name=f"wq{kc}", tag=f"wq{kc}")
                    for kc in range(4)]
            wk_t = [hp.tile([128, E], bf16, name=f"wk{kc}", tag=f"wk{kc}")
                    for kc in range(4)]
            for kc in range(4):
                nc.gpsimd.dma_start(
                    wq_t[kc], wq_d[kc * 128:(kc + 1) * 128, hof:hof + E])
                nc.gpsimd.dma_start(
                    wk_t[kc], wk_d[kc * 128:(kc + 1) * 128, hof:hof + E])
            bqc = hp.tile([128, 4], f32, tag="bqc")
            nc.sync.dma_start(
                bqc, bq_d[0:1, hof:hof + E].rearrange("o (c p) -> (o p) c",
                                                      p=128))
            bkc = hp.tile([128, 4], f32, tag="bkc")
            nc.sync.dma_start(
                bkc, bk_d[0:1, hof:hof + E].rearrange("o (c p) -> (o p) c",
                                                      p=128))
            return wq_t, wk_t, bqc, bkc

        # ---- Phase 1: LN1 -> y^T ----
        PRE0 = emit_qk_dma(0)
        layernorm_transpose(None, sc1, bi1, yT, "ln1", cast16=yT16)

        # Deferred AdaLN is emitted from inside the head loop (see below) so
        # its weight DMAs stay off the phase-0/head-0 DMA critical path.
        MOD = {}

        def emit_adaln_a1_seed():
            MOD["A1"] = adaln_rep("a1")
            # y = x + (o@lvw + lvb)*a1 accumulated head by head with a1
            # folded into lvw; seed the accumulator with x + lvb*a1.
            LVBA = const.tile([128, E], f32)
            nc.sync.dma_start(LVBA, lvb_d.broadcast_to([128, E]))
            nc.vector.tensor_mul(LVBA, LVBA, MOD["A1"])
            for t in range(8):
                x_t3 = work.tile([128, E], f32, name="xt3", tag="wbig")
                nc.sync.dma_start(x_t3, x_d[t * 128:(t + 1) * 128, :])
                nc.vector.tensor_add(y2acc[t], x_t3, LVBA)

        def emit_adaln_ln2():
            MOD["sc2"], MOD["bi2"] = scale_shift(
                1, adaln_cols("g2"), adaln_cols("be2"), ln2g_d, ln2b_d)
            MOD["A2"] = adaln_rep("a2")

        # LN2 emitted per tile from inside the last head's lv loop, so its
        # DVE chain hides under the remaining lv matmuls.
        zT = [const.tile([128, S], f32r, name=f"zT{c}", tag=f"yT{c}")
              for c in range(4)]

        _ln2_pend = []

        def ln2_tile(t):
            _ln2_pend.append((t, ln_stats(y2acc[t], "ln2")))
            if len(_ln2_pend) < 2:
                return
            (ta, xa), (tb, xb) = _ln2_pend
            _ln2_pend.clear()
            assert tb == ta + 1
            for ec in range(4):
                tp2 = psum_tp.tile([128, 256], f32r, name="tp_ln2", tag="ptp")
                nc.tensor.matmul(tp2[:, 0:128], xa[:, ec * 128:(ec + 1) * 128],
                                 ident, is_transpose=True, start=True, stop=False)
                nc.tensor.matmul(tp2[:, 128:256], xb[:, ec * 128:(ec + 1) * 128],
                                 ident, is_transpose=True, start=False, stop=True)
                nc.scalar.activation(
                    zT[ec][:, ta * 128:(ta + 2) * 128], tp2, AF.Identity,
                    bias=MOD["bi2"][:, ec:ec + 1], scale=MOD["sc2"][:, ec:ec + 1])

        LN2_CB = [ln2_tile]

        # ---- Phase 2: attention heads (software-pipelined) ----
        if True:

            def head_qk(h, pre=None):
                """Compute Q^T, K^T for head h."""
                wq_t, wk_t, bqc, bkc = pre if pre is not None else emit_qk_dma(h)
                QT = [hp.tile([128, S], bf16, name=f"QT{mc}", tag=f"QT{mc}")
                      for mc in range(4)]
                KT = [hp.tile([128, S], bf16, name=f"KT{mc}", tag=f"KT{mc}")
                      for mc in range(4)]
                for mc in range(4):
                    for sh in range(2):
                        pq = psum_mm.tile([128, 512], f32, name="pq", tag="pmm")
                        for kc in range(4):
                            nc.tensor.matmul(
                                pq, wq_t[kc][:, mc * 128:(mc + 1) * 128],
                                yT16[kc][:, sh * 512:(sh + 1) * 512],
                                start=(kc == 0), stop=(kc == 3))
                        nc.vector.tensor_scalar_add(
                            QT[mc][:, sh * 512:(sh + 1) * 512], pq,
                            bqc[:, mc:mc + 1])
                        pk = psum_mm.tile([128, 512], f32, name="pk", tag="pmm")
                        for kc in range(4):
                            nc.tensor.matmul(
                                pk, wk_t[kc][:, mc * 128:(mc + 1) * 128],
                                yT16[kc][:, sh * 512:(sh + 1) * 512],
                                start=(kc == 0), stop=(kc == 3))
                        nc.vector.tensor_scalar_add(
                            KT[mc][:, sh * 512:(sh + 1) * 512], pk,
                            bkc[:, mc:mc + 1])
                return QT, KT

            def head_v(h):
                """Load wv slice, compute V (natural layout) for head h."""
                hof = h * E
                wv_t = [hp.tile([128, E], f32r, name=f"wv{kc}", tag=f"wv{kc}")
                        for kc in range(4)]
                for kc in range(4):
                    nc.sync.dma_start(
                        wv_t[kc], wv_d[kc * 128:(kc + 1) * 128, hof:hof + E])
                BVrep = hp.tile([128, E], f32, tag="bvrep")
                nc.sync.dma_start(
                    BVrep, bv_d[0:1, hof:hof + E].broadcast_to([128, E]))
                Vh = [hp.tile([128, E], bf16, name=f"V{tc_}", tag=f"V{tc_}")
                      for tc_ in range(8)]
                for tc_ in range(8):
                    pv = psum_mm.tile([128, 512], f32, name="pv", tag="pmm")
                    for kc in range(4):
                        nc.tensor.matmul(
                            pv, yT[kc][:, tc_ * 128:(tc_ + 1) * 128], wv_t[kc],
                            start=(kc == 0), stop=(kc == 3))
                    nc.vector.tensor_add(Vh[tc_], pv, BVrep)
                return Vh

            def head_scores(QT, KT):
                """scores^T + exp; incremental DVE tree for denominators."""
                Eh = [hp.tile([128, S], bf16, name=f"E{tc_}", tag=f"E{tc_}")
                      for tc_ in range(8)]
                esum = hp.tile([128, S], f32, tag="esum")
                for tc_ in range(8):
                    for sh in range(2):
                        ps = psum_mm.tile([128, 512], f32, name="ps", tag="pmm")
                        for ec in range(4):
                            nc.tensor.matmul(
                                ps, KT[ec][:, tc_ * 128:(tc_ + 1) * 128],
                                QT[ec][:, sh * 512:(sh + 1) * 512],
                                start=(ec == 0), stop=(ec == 3))
                        nc.scalar.activation(
                            Eh[tc_][:, sh * 512:(sh + 1) * 512], ps, AF.Exp,
                            scale=SCALE)
                    if tc_ == 1:
                        nc.vector.tensor_add(esum, Eh[0], Eh[1])
                    elif tc_ > 1:
                        nc.vector.tensor_add(esum, esum, Eh[tc_])
                return Eh, esum

            def head_sums(esum):
                """Softmax denominators -> replicated reciprocal rows.
                Copy PSUM rows out fast (frees the bank), broadcast, then
                take the reciprocal across all 128 lanes."""
                srow = hp.tile([1, S], f32, tag="srow")
                for sh in range(2):
                    psr = psum_row.tile([1, 512], f32, name="psr", tag="prow")
                    nc.tensor.matmul(psr, ones_col,
                                     esum[:, sh * 512:(sh + 1) * 512],
                                     start=True, stop=True)
                    nc.vector.tensor_copy(srow[0:1, sh * 512:(sh + 1) * 512], psr)
                Srep = hp.tile([128, S], f32, tag="esum")
                nc.gpsimd.partition_broadcast(Srep, srow)
                Rrep = hp.tile([128, S], f32, tag="rrep")
                nc.vector.reciprocal_approx_fast(Rrep, Srep)
                return Rrep

            def head_av(Vh, Eh, Rrep):
                """AV matmuls; 1/sum applied on the PSUM->SBUF move."""
                oT = [hp.tile([128, S], f32r, name=f"oT{ec}", tag=f"oT{ec}")
                      for ec in range(4)]
                for ec in range(4):
                    for sh in range(2):
                        po = psum_mm.tile([128, 512], f32, name="po", tag="pmm")
                        for tc_ in range(8):
                            nc.tensor.matmul(
                                po, Vh[tc_][:, ec * 128:(ec + 1) * 128],
                                Eh[tc_][:, sh * 512:(sh + 1) * 512],
                                start=(tc_ == 0), stop=(tc_ == 7))
                        nc.vector.tensor_mul(
                            oT[ec][:, sh * 512:(sh + 1) * 512], po,
                            Rrep[:, sh * 512:(sh + 1) * 512])
                return oT

            def head_lv(h, oT, ln2_cb=None):
                """lv partial accumulate (alpha1 pre-folded into lvw)."""
                hof = h * E
                lvw_t = [hp.tile([128, E], f32r, name=f"lvw{kc}", tag=f"lvw{kc}")
                         for kc in range(4)]
                for kc in range(4):
                    nc.sync.dma_start(
                        lvw_t[kc], lvw_d[hof + kc * 128:hof + (kc + 1) * 128, :])
                    nc.vector.tensor_mul(lvw_t[kc], lvw_t[kc], MOD["A1"])
                for t in range(8):
                    py = psum_mm.tile([128, 512], f32, name="py", tag="pmm")
                    for kc in range(4):
                        nc.tensor.matmul(
                            py, oT[kc][:, t * 128:(t + 1) * 128], lvw_t[kc],
                            start=(kc == 0), stop=(kc == 3))
                    nc.vector.tensor_add(y2acc[t], y2acc[t], py)
                    if ln2_cb is not None:
                        ln2_cb(t)

            # Pipelined loop. PE stream per iteration:
            #   QK(h) -> sums(h-1) -> AV(h-1) -> V(h) -> scores(h) -> lv(h-1)
            # so exp/tree/recip of h-1 all hide under dense PE work.
            prev = None
            prev_oT = None
            for h in range(H):
                QT, KT = head_qk(h, pre=(PRE0 if h == 0 else None))
                if prev is not None:
                    Rrep = head_sums(prev[3])
                    prev_oT = head_av(prev[1], prev[2], Rrep)
                Vh = head_v(h)
                Eh, esum = head_scores(QT, KT)
                if prev is not None:
                    head_lv(prev[0], prev_oT)
                if h == 0:
                    emit_adaln_a1_seed()
                elif h == 2:
                    emit_adaln_ln2()
                elif h >= 4:
                    emit_f1w_quarter(h - 4)
                prev = (h, Vh, Eh, esum)
            Rrep = head_sums(prev[3])
            prev_oT = head_av(prev[1], prev[2], Rrep)
            head_lv(prev[0], prev_oT, ln2_cb=LN2_CB[0])
        hp_ctx.close()

        # ---- Phase 5 prefetch: FFN weights (hidden under residual/LN2) ----
        fp = ctx.enter_context(tc.tile_pool(name="fp", bufs=1))

        # keep the PE clock-gate open across the LN2/weight-DMA boundary
        for i in range(12):
            warm2 = psum_mm.tile([128, 512], f32, name="warm2", tag="pmm")
            nc.tensor.matmul(warm2[:, 0:128], ident, ident,
                             start=True, stop=True)

        # ---- Phase 5: FFN ----
        hT = [fp.tile([128, S], f32r, name=f"hT{hc}", tag=f"hT{hc}")
              for hc in range(16)]
        for sh in range(2):
            for hc in range(16):
                pf = psum_mm.tile([128, 512], f32, name="pf", tag="pmm")
                for kc in range(4):
                    nc.tensor.matmul(
                        pf, f1w_t[kc][:, hc * 128:(hc + 1) * 128],
                        zT[kc][:, sh * 512:(sh + 1) * 512],
                        start=(kc == 0), stop=(kc == 3))
                nc.scalar.activation(
                    hT[hc][:, sh * 512:(sh + 1) * 512], pf, AF.Relu,
                    bias=f1bc[:, hc:hc + 1], scale=1.0)
        f2w_t = [fp.tile([128, E], f32r, name=f"f2w{kc}", tag=f"f2w{kc}")
                 for kc in range(16)]
        for kc in range(16):
            nc.sync.dma_start(f2w_t[kc], f2w_d[kc * 128:(kc + 1) * 128, :])
        F2B_rep = fp.tile([128, E], f32, tag="f2brep")
        nc.sync.dma_start(F2B_rep, f2b_d.broadcast_to([128, E]))
        for t in range(8):
            pz = psum_mm.tile([128, 512], f32, name="pz", tag="pmm")
            for kc in range(16):
                nc.tensor.matmul(
                    pz, hT[kc][:, t * 128:(t + 1) * 128], f2w_t[kc],
                    start=(kc == 0), stop=(kc == 15))
            q1 = work.tile([128, E], f32, name="q1", tag="wbig")
            nc.vector.tensor_add(q1, pz, F2B_rep)
            nc.vector.tensor_mul(q1, q1, MOD["A2"])
            ot = work.tile([128, E], f32, name="ot", tag="wbig")
            nc.vector.tensor_add(ot, q1, y2acc[t])
            nc.sync.dma_start(out_d[t * 128:(t + 1) * 128, :], ot)

    nc.compile()
    return nc


def _get_program():
    if "nc" not in _CACHE:
        _CACHE["nc"] = _build()
    return _CACHE["nc"]


def kernel(**inputs) -> np.ndarray:
    from concourse.bass_utils import run_bass_kernel_spmd

    ins = {k: np.asarray(v, dtype=np.float32) for k, v in inputs.items()}
    nc = _get_program()

    in_maps = []
    for b in range(B):
        m = {
            "x": ins["x"][b],                       # (S, E)
            "cond": ins["cond"][b].reshape(E, 1),   # (E, 1)
            "ln1g": ins["ln1g"].reshape(1, E), "ln1b": ins["ln1b"].reshape(1, E),
            "ln2g": ins["ln2g"].reshape(1, E), "ln2b": ins["ln2b"].reshape(1, E),
            "wq": ins["wq"], "wk": ins["wk"], "wv": ins["wv"],
            "bq": ins["bq"].reshape(1, HE), "bk": ins["bk"].reshape(1, HE),
            "bv": ins["bv"].reshape(1, HE),
            "lvw": ins["lvw"], "lvb": ins["lvb"].reshape(1, E),
            "f1w": ins["f1w"], "f1b": ins["f1b"].reshape(1, FF),
            "f2w": ins["f2w"], "f2b": ins["f2b"].reshape(1, E),
            "ident": np.eye(128, dtype=np.float32),
        }
        for nm in ["g1", "be1", "a1", "g2", "be2", "a2"]:
            m[f"{nm}w"] = ins[f"{nm}w"]
            m[f"{nm}b"] = ins[f"{nm}b"].reshape(1, E)
        in_maps.append(m)

    res = run_bass_kernel_spmd(nc, in_maps, list(range(N_CORES)),
                               trace=TRACE, tmpdir=TRACE_DIR)
    _CACHE["last_result"] = res
    out = np.stack([res.results[b]["out"] for b in range(B)], axis=0)
    return out



# revision 12
# speedup vs baseline: 1.1198x; 1.1198x over previous
"""DiT block kernel for Trainium2, data-parallel over batch across 8 NeuronCores.

Problem: nn_DiTBlock — B=8, S=1024, E=512, H=8 (head_dim = E = 512).
Sharding: batch element b -> core b. Each core runs the full DiT block on its
(S, E) slice with replicated weights; no collectives.

Per-core dataflow (activations kept transposed so the contraction dim sits on
partitions; float32r matmuls = full-rate PE with ~1.5e-4 relative rounding):
  AdaLN matvecs (PE) -> LN1 stats (DVE bn_stats) -> PE-transpose with fused
  modulate (ACT per-partition scale/bias) -> y^T
  -> software-pipelined head loop: QK(h) emitted between scores(h-1) and
     AV(h-1) so exp/tree latency hides under PE work
  -> residual -> LN2 (same transpose-modulate trick) -> FFN (relu in ACT)
  -> out = y + (h @ f2w + f2b) * alpha2.
"""
import sys
import numpy as np

sys.path.insert(0, '/opt/trn_rl_repo')

B, S, E, H = 8, 1024, 512, 8
HE = H * E          # 4096
FF = 4 * E          # 2048
EPS = 1e-5
SCALE = 1.0 / 32.0  # 1/sqrt(S)
N_CORES = 8

# fp8 (e4m3) quantization scales for the scores/AV DoubleRow path.
# Ranges (empirical max over the reference inputs, 4x+ margin to the 240
# clip): |Q|,|K| <= ~3.1 -> x16 = 49; exp(scores/32) <= ~3.3 -> x16 = 53;
# |V| <= ~3.2 -> x32 = 100.  |o| <= max|V| (convex combination).
S_QK = 16.0
S_EXP = 16.0
S_V = 32.0
import math as _math
EXP_BIAS = _math.log(S_EXP)           # exp(x)*S_EXP = exp(x + ln S_EXP)
SCL_SCORES = SCALE / (S_QK * S_QK)    # dequant of QT8.KT8 psum into exp

TRACE = False       # set by test harness to capture an NTFF profile
TRACE_DIR = None

_CACHE = {}


def _build():
    from contextlib import ExitStack
    import concourse.bass as bass
    import concourse.tile as tile
    from concourse import bacc, mybir
    f32 = mybir.dt.float32
    f32r = mybir.dt.float32r
    bf16 = mybir.dt.bfloat16
    fp8 = mybir.dt.float8e4
    PM = mybir.MatmulPerfMode.DoubleRow
    AF = mybir.ActivationFunctionType
    ALU = mybir.AluOpType

    nc = bacc.Bacc("TRN2", target_bir_lowering=False, debug=False,
                   num_devices=N_CORES)

    # ---- DRAM parameters --------------------------------------------------
    x_d = nc.dram_tensor("x", [S, E], f32, kind="ExternalInput").ap()
    cond_d = nc.dram_tensor("cond", [E, 1], f32, kind="ExternalInput").ap()

    adaln_w = {}
    adaln_b = {}
    for nm in ["g1", "be1", "a1", "g2", "be2", "a2"]:
        adaln_w[nm] = nc.dram_tensor(f"{nm}w", [E, E], f32,
                                     kind="ExternalInput").ap()
        adaln_b[nm] = nc.dram_tensor(f"{nm}b", [1, E], f32,
                                     kind="ExternalInput").ap()
    ln1g_d = nc.dram_tensor("ln1g", [1, E], f32, kind="ExternalInput").ap()
    ln1b_d = nc.dram_tensor("ln1b", [1, E], f32, kind="ExternalInput").ap()
    ln2g_d = nc.dram_tensor("ln2g", [1, E], f32, kind="ExternalInput").ap()
    ln2b_d = nc.dram_tensor("ln2b", [1, E], f32, kind="ExternalInput").ap()
    wq_d = nc.dram_tensor("wq", [E, HE], f32r, kind="ExternalInput").ap()
    wk_d = nc.dram_tensor("wk", [E, HE], f32r, kind="ExternalInput").ap()
    wv_d = nc.dram_tensor("wv", [E, HE], f32r, kind="ExternalInput").ap()
    bq_d = nc.dram_tensor("bq", [1, HE], f32, kind="ExternalInput").ap()
    # bk pre-scaled by S_QK on host (folded into the KT8 quantize ACT);
    # bv is folded into lvb on host (o = softmax-avg of V is bias-affine).
    bk_d = nc.dram_tensor("bk_s", [1, HE], f32, kind="ExternalInput").ap()
    lvw_d = nc.dram_tensor("lvw", [HE, E], f32r, kind="ExternalInput").ap()
    lvb_d = nc.dram_tensor("lvb", [1, E], f32, kind="ExternalInput").ap()
    f1w_d = nc.dram_tensor("f1w", [E, FF], f32r, kind="ExternalInput").ap()
    f1b_d = nc.dram_tensor("f1b", [1, FF], f32, kind="ExternalInput").ap()
    f2w_d = nc.dram_tensor("f2w", [FF, E], f32r, kind="ExternalInput").ap()
    f2b_d = nc.dram_tensor("f2b", [1, E], f32, kind="ExternalInput").ap()
    ident_d = nc.dram_tensor("ident", [128, 128], f32r,
                             kind="ExternalInput").ap()
    out_d = nc.dram_tensor("out", [S, E], f32, kind="ExternalOutput").ap()

    with tile.TileContext(nc) as tc, ExitStack() as ctx:
        const = ctx.enter_context(tc.tile_pool(name="const", bufs=1))
        work = ctx.enter_context(tc.tile_pool(name="work", bufs=3))
        psum_mm = ctx.enter_context(
            tc.tile_pool(name="psum_mm", bufs=5, space="PSUM"))
        psum_tp = ctx.enter_context(
            tc.tile_pool(name="psum_tp", bufs=2, space="PSUM"))
        psum_row = ctx.enter_context(
            tc.tile_pool(name="psum_row", bufs=1, space="PSUM"))

        # ---- constants ----
        ident = const.tile([128, 128], f32r)
        nc.sync.dma_start(ident, ident_d)
        eps_t = const.tile([128, 1], f32)
        nc.vector.memset(eps_t, EPS)
        # stationary column for the softmax-denominator matvec; its value
        # S_V folds V's fp8 dequant into the reciprocal (oT = po * Rrep).
        ones_col = const.tile([128, 1], f32)
        nc.vector.memset(ones_col, S_V)
        expb_t = const.tile([128, 1], f32)
        nc.vector.memset(expb_t, EXP_BIAS)

        cond_cols = const.tile([128, 4], f32)
        nc.sync.dma_start(cond_cols, cond_d.rearrange("(c p) o -> p (c o)", p=128))

        adp = ctx.enter_context(tc.tile_pool(name="adp", bufs=2))

        def adaln_cols(nm):
            """(cond @ W + b) laid out as [128, 4] e-columns (PE matvec)."""
            pcol = psum_tp.tile([128, 4], f32, name=f"pcol_{nm}", tag="ptp")
            for kc in range(4):
                adw = adp.tile([128, E], f32, name=f"adw_{nm}_{kc}", tag="adw")
                nc.sync.dma_start(adw, adaln_w[nm][kc * 128:(kc + 1) * 128, :])
                for ec in range(4):
                    nc.tensor.matmul(
                        pcol[:, ec:ec + 1],
                        adw[:, ec * 128:(ec + 1) * 128],
                        cond_cols[:, kc:kc + 1],
                        start=(kc == 0 and ec == 0),
                        stop=(kc == 3 and ec == 3))
            bcol = adp.tile([128, 4], f32, name=f"bcol_{nm}", tag="bcol")
            nc.sync.dma_start(
                bcol, adaln_b[nm].rearrange("o (c p) -> (o p) c", p=128))
            mcol = const.tile([128, 4], f32, name=f"mcol_{nm}")
            nc.vector.tensor_add(mcol, pcol, bcol)
            return mcol

        def adaln_rep(nm):
            """(cond @ W + b) replicated to [128, E] (row matvec + bcast)."""
            prow = psum_row.tile([1, E], f32, name=f"prow_{nm}", tag="prow")
            for kc in range(4):
                adw = adp.tile([128, E], f32, name=f"adw_{nm}_{kc}", tag="adw")
                nc.sync.dma_start(adw, adaln_w[nm][kc * 128:(kc + 1) * 128, :])
                nc.tensor.matmul(prow, cond_cols[:, kc:kc + 1], adw,
                                 start=(kc == 0), stop=(kc == 3))
            brow = adp.tile([1, E], f32, name=f"brow_{nm}", tag="brow")
            nc.sync.dma_start(brow, adaln_b[nm])
            arow = adp.tile([1, E], f32, name=f"arow_{nm}", tag="arow")
            nc.vector.tensor_add(arow, prow, brow)
            arep = const.tile([128, E], f32, name=f"arep_{nm}")
            nc.gpsimd.partition_broadcast(arep, arow)
            return arep

        def scale_shift(li, gcol, bcol, lng_d, lnb_d):
            """sc = ln_g*(1+gamma), bi = ln_b*(1+gamma)+beta, as [128,4] cols."""
            gp = const.tile([128, 4], f32, name=f"gp_{li}")
            nc.vector.tensor_scalar_add(gp, gcol, 1.0)
            lgc = adp.tile([128, 4], f32, name=f"lgc_{li}", tag="lgc")
            nc.sync.dma_start(lgc, lng_d.rearrange("o (c p) -> (o p) c", p=128))
            lbc = adp.tile([128, 4], f32, name=f"lbc_{li}", tag="lbc")
            nc.sync.dma_start(lbc, lnb_d.rearrange("o (c p) -> (o p) c", p=128))
            sc = const.tile([128, 4], f32, name=f"sc_{li}")
            nc.vector.tensor_mul(sc, lgc, gp)
            bi = const.tile([128, 4], f32, name=f"bi_{li}")
            nc.vector.tensor_mul(bi, lbc, gp)
            nc.vector.tensor_add(bi, bi, bcol)
            return sc, bi

        # Only g1/be1 gate the LN1->y^T critical path; defer the rest.
        sc1, bi1 = scale_shift(0, adaln_cols("g1"), adaln_cols("be1"),
                               ln1g_d, ln1b_d)

        # ---- persistent activation tiles ----
        yT = [const.tile([128, S], f32r, name=f"yT{c}") for c in range(4)]
        yT16 = [const.tile([128, S], bf16, name=f"yT16_{c}") for c in range(4)]
        y2acc = [const.tile([128, E], f32, name=f"y2acc{t}") for t in range(8)]

        def ln_stats(x_t, tagp):
            st = work.tile([128, 6], f32, name=f"st_{tagp}", tag=f"st_{tagp}")
            nc.vector.bn_stats(st, x_t)
            mv = work.tile([128, 2], f32, name=f"mv_{tagp}", tag=f"mv_{tagp}")
            nc.vector.bn_aggr(mv, st)
            rs = work.tile([128, 1], f32, name=f"rs_{tagp}", tag=f"rs_{tagp}")
            nc.scalar.activation(rs, mv[:, 1:2], AF.Sqrt, bias=eps_t, scale=1.0)
            nc.vector.reciprocal(rs, rs)
            xn = work.tile([128, E], f32r, name=f"xn_{tagp}", tag="wxn")
            nc.vector.tensor_scalar(xn, x_t, scalar1=mv[:, 0:1], scalar2=rs,
                                    op0=ALU.subtract, op1=ALU.mult)
            return xn

        def layernorm_transpose(src_tiles, scol, bcol, dst_T, tagp,
                                cast16=None):
            """LN over free dim of [128, E] tiles; PE-transpose 128-blocks in
            pairs sharing one PSUM bank; one fused modulate ACT per pair.
            (Only the pair's first transpose sets start=True: start clears the
            whole bank.)"""
            for t0 in range(0, 8, 2):
                xns = []
                for t in (t0, t0 + 1):
                    if src_tiles is None:
                        x_t = work.tile([128, E], f32, name=f"xt_{tagp}",
                                        tag="wbig")
                        nc.sync.dma_start(x_t, x_d[t * 128:(t + 1) * 128, :])
                    else:
                        x_t = src_tiles[t]
                    xns.append(ln_stats(x_t, tagp))
                for ec in range(4):
                    tp = psum_tp.tile([128, 256], f32r, name=f"tp_{tagp}",
                                      tag="ptp")
                    nc.tensor.matmul(tp[:, 0:128],
                                     xns[0][:, ec * 128:(ec + 1) * 128], ident,
                                     is_transpose=True, start=True, stop=False)
                    nc.tensor.matmul(tp[:, 128:256],
                                     xns[1][:, ec * 128:(ec + 1) * 128], ident,
                                     is_transpose=True, start=False, stop=True)
                    nc.scalar.activation(
                        dst_T[ec][:, t0 * 128:(t0 + 2) * 128], tp, AF.Identity,
                        bias=bcol[:, ec:ec + 1], scale=scol[:, ec:ec + 1])
                if cast16 is not None:
                    for c in range(4):
                        nc.vector.tensor_copy(
                            cast16[c][:, t0 * 128:(t0 + 2) * 128],
                            dst_T[c][:, t0 * 128:(t0 + 2) * 128])

        # f1w prefetched in quarter-chunks during heads 4-7 (small DMAs so
        # the sync queue never head-of-line blocks the per-head weights).
        ffp = ctx.enter_context(tc.tile_pool(name="ffp", bufs=1))
        f1w_t = [ffp.tile([128, FF], f32r, name=f"f1w{kc}", tag=f"f1w{kc}")
                 for kc in range(4)]
        f1bc = ffp.tile([128, 16], f32, tag="f1bc")

        def emit_f1w_quarter(q):
            for kc in range(4):
                nc.sync.dma_start(
                    f1w_t[kc][:, q * 512:(q + 1) * 512],
                    f1w_d[kc * 128:(kc + 1) * 128, q * 512:(q + 1) * 512])
            if q == 3:
                nc.sync.dma_start(
                    f1bc, f1b_d.rearrange("o (c p) -> (o p) c", p=128))

        hp_ctx = ExitStack()
        hp = hp_ctx.enter_context(tc.tile_pool(name="hp", bufs=1))

        def emit_qk_dma(h):
            hof = h * E
            wq_t = [hp.tile([128, E], bf16, name=f"wq{kc}", tag=f"wq{kc}")
                    for kc in range(4)]
            wk_t = [hp.tile([128, E], bf16, name=f"wk{kc}", tag=f"wk{kc}")
                    for kc in range(4)]
            for kc in range(4):
                nc.gpsimd.dma_start(
                    wq_t[kc], wq_d[kc * 128:(kc + 1) * 128, hof:hof + E])
                nc.gpsimd.dma_start(
                    wk_t[kc], wk_d[kc * 128:(kc + 1) * 128, hof:hof + E])
            bqc = hp.tile([128, 4], f32, tag="bqc")
            nc.sync.dma_start(
                bqc, bq_d[0:1, hof:hof + E].rearrange("o (c p) -> (o p) c",
                                                      p=128))
            bkc = hp.tile([128, 4], f32, tag="bkc")
            nc.sync.dma_start(
                bkc, bk_d[0:1, hof:hof + E].rearrange("o (c p) -> (o p) c",
                                                      p=128))
            return wq_t, wk_t, bqc, bkc

        # ---- Phase 1: LN1 -> y^T ----
        PRE0 = emit_qk_dma(0)
        layernorm_transpose(None, sc1, bi1, yT, "ln1", cast16=yT16)

        # Deferred AdaLN is emitted from inside the head loop (see below) so
        # its weight DMAs stay off the phase-0/head-0 DMA critical path.
        MOD = {}

        def emit_adaln_a1_seed():
            MOD["A1"] = adaln_rep("a1")
            # y = x + (o@lvw + lvb)*a1 accumulated head by head with a1
            # folded into lvw; seed the accumulator with x + lvb*a1.
            LVBA = const.tile([128, E], f32)
            nc.sync.dma_start(LVBA, lvb_d.broadcast_to([128, E]))
            nc.vector.tensor_mul(LVBA, LVBA, MOD["A1"])
            for t in range(8):
                x_t3 = work.tile([128, E], f32, name="xt3", tag="wbig")
                nc.sync.dma_start(x_t3, x_d[t * 128:(t + 1) * 128, :])
                nc.vector.tensor_add(y2acc[t], x_t3, LVBA)

        def emit_adaln_ln2():
            MOD["sc2"], MOD["bi2"] = scale_shift(
                1, adaln_cols("g2"), adaln_cols("be2"), ln2g_d, ln2b_d)
            MOD["A2"] = adaln_rep("a2")

        # LN2 emitted per tile from inside the last head's lv loop, so its
        # DVE chain hides under the remaining lv matmuls.
        zT = [const.tile([128, S], f32r, name=f"zT{c}", tag=f"yT{c}")
              for c in range(4)]

        _ln2_pend = []

        def ln2_tile(t):
            _ln2_pend.append((t, ln_stats(y2acc[t], "ln2")))
            if len(_ln2_pend) < 2:
                return
            (ta, xa), (tb, xb) = _ln2_pend
            _ln2_pend.clear()
            assert tb == ta + 1
            for ec in range(4):
                tp2 = psum_tp.tile([128, 256], f32r, name="tp_ln2", tag="ptp")
                nc.tensor.matmul(tp2[:, 0:128], xa[:, ec * 128:(ec + 1) * 128],
                                 ident, is_transpose=True, start=True, stop=False)
                nc.tensor.matmul(tp2[:, 128:256], xb[:, ec * 128:(ec + 1) * 128],
                                 ident, is_transpose=True, start=False, stop=True)
                nc.scalar.activation(
                    zT[ec][:, ta * 128:(ta + 2) * 128], tp2, AF.Identity,
                    bias=MOD["bi2"][:, ec:ec + 1], scale=MOD["sc2"][:, ec:ec + 1])

        LN2_CB = [ln2_tile]

        # ---- Phase 2: attention heads (software-pipelined) ----
        if True:

            def head_qk(h, pre=None):
                """Compute Q^T, K^T for head h, quantized to fp8 in the
                DoubleRow pair layout [128, 2, S] (pair = mc//2)."""
                wq_t, wk_t, bqc, bkc = pre if pre is not None else emit_qk_dma(h)
                QT = [hp.tile([128, 2, S], fp8, name=f"QT8_{g}", tag=f"QT{g}")
                      for g in range(2)]
                KT = [hp.tile([128, 2, S], fp8, name=f"KT8_{g}", tag=f"KT{g}")
                      for g in range(2)]
                for mc in range(4):
                    g, i = mc // 2, mc % 2
                    for sh in range(2):
                        pq = psum_mm.tile([128, 512], f32, name="pq", tag="pmm")
                        for kc in range(4):
                            nc.tensor.matmul(
                                pq, wq_t[kc][:, mc * 128:(mc + 1) * 128],
                                yT16[kc][:, sh * 512:(sh + 1) * 512],
                                start=(kc == 0), stop=(kc == 3))
                        nc.vector.tensor_scalar(
                            QT[g][:, i, sh * 512:(sh + 1) * 512], pq,
                            scalar1=bqc[:, mc:mc + 1], scalar2=S_QK,
                            op0=ALU.add, op1=ALU.mult)
                        pk = psum_mm.tile([128, 512], f32, name="pk", tag="pmm")
                        for kc in range(4):
                            nc.tensor.matmul(
                                pk, wk_t[kc][:, mc * 128:(mc + 1) * 128],
                                yT16[kc][:, sh * 512:(sh + 1) * 512],
                                start=(kc == 0), stop=(kc == 3))
                        # bkc is host-prescaled by S_QK: (pk*S_QK + bk*S_QK)
                        nc.scalar.activation(
                            KT[g][:, i, sh * 512:(sh + 1) * 512], pk,
                            AF.Identity, bias=bkc[:, mc:mc + 1], scale=S_QK)
                return QT, KT

            def head_v(h):
                """Load wv slice, compute V*S_V as fp8 pairs [128, 2, E]
                (pair = tc//2).  bv is folded into lvb on the host."""
                hof = h * E
                wv_t = [hp.tile([128, E], f32r, name=f"wv{kc}", tag=f"wv{kc}")
                        for kc in range(4)]
                for kc in range(4):
                    nc.sync.dma_start(
                        wv_t[kc], wv_d[kc * 128:(kc + 1) * 128, hof:hof + E])
                Vh = [hp.tile([128, 2, E], fp8, name=f"V8_{g}", tag=f"V{g}")
                      for g in range(4)]
                for tc_ in range(8):
                    pv = psum_mm.tile([128, 512], f32, name="pv", tag="pmm")
                    for kc in range(4):
                        nc.tensor.matmul(
                            pv, yT[kc][:, tc_ * 128:(tc_ + 1) * 128], wv_t[kc],
                            start=(kc == 0), stop=(kc == 3))
                    nc.scalar.activation(
                        Vh[tc_ // 2][:, tc_ % 2, :], pv, AF.Identity,
                        scale=S_V)
                return Vh

            def head_scores(QT, KT):
                """scores^T via fp8 DoubleRow + exp*S_EXP into fp8 pairs
                [128, 2, S] (pair = tc//2); DVE tree for denominators."""
                Eh = [hp.tile([128, 2, S], fp8, name=f"E8_{g}", tag=f"E{g}")
                      for g in range(4)]
                esum = hp.tile([128, S], f32, tag="esum")
                for tc_ in range(8):
                    for sh in range(2):
                        ps = psum_mm.tile([128, 512], f32, name="ps", tag="pmm")
                        for g in range(2):
                            nc.tensor.matmul(
                                ps, KT[g][:, :, tc_ * 128:(tc_ + 1) * 128],
                                QT[g][:, :, sh * 512:(sh + 1) * 512],
                                start=(g == 0), stop=(g == 1), perf_mode=PM)
                        nc.scalar.activation(
                            Eh[tc_ // 2][:, tc_ % 2, sh * 512:(sh + 1) * 512],
                            ps, AF.Exp, scale=SCL_SCORES, bias=expb_t)
                    if tc_ == 1:
                        nc.vector.tensor_add(
                            esum, Eh[0][:, 0, :], Eh[0][:, 1, :])
                    elif tc_ > 1:
                        nc.vector.tensor_add(
                            esum, esum, Eh[tc_ // 2][:, tc_ % 2, :])
                return Eh, esum

            def head_sums(esum):
                """Softmax denominators -> replicated reciprocal rows.
                Copy PSUM rows out fast (frees the bank), broadcast, then
                take the reciprocal across all 128 lanes."""
                srow = hp.tile([1, S], f32, tag="srow")
                for sh in range(2):
                    psr = psum_row.tile([1, 512], f32, name="psr", tag="prow")
                    nc.tensor.matmul(psr, ones_col,
                                     esum[:, sh * 512:(sh + 1) * 512],
                                     start=True, stop=True)
                    nc.vector.tensor_copy(srow[0:1, sh * 512:(sh + 1) * 512], psr)
                Srep = hp.tile([128, S], f32, tag="esum")
                nc.gpsimd.partition_broadcast(Srep, srow)
                Rrep = hp.tile([128, S], f32, tag="rrep")
                nc.vector.reciprocal_approx_fast(Rrep, Srep)
                return Rrep

            def head_av(Vh, Eh, Rrep):
                """AV via fp8 DoubleRow; 1/sum (with fp8 dequants folded via
                the ones_col=S_V trick) applied on the PSUM->SBUF move."""
                oT = [hp.tile([128, S], f32r, name=f"oT{ec}", tag=f"oT{ec}")
                      for ec in range(4)]
                for ec in range(4):
                    for sh in range(2):
                        po = psum_mm.tile([128, 512], f32, name="po", tag="pmm")
                        for g in range(4):
                            nc.tensor.matmul(
                                po, Vh[g][:, :, ec * 128:(ec + 1) * 128],
                                Eh[g][:, :, sh * 512:(sh + 1) * 512],
                                start=(g == 0), stop=(g == 3), perf_mode=PM)
                        nc.vector.tensor_mul(
                            oT[ec][:, sh * 512:(sh + 1) * 512], po,
                            Rrep[:, sh * 512:(sh + 1) * 512])
                return oT

            def head_lv(h, oT, ln2_cb=None):
                """lv partial accumulate (alpha1 pre-folded into lvw)."""
                hof = h * E
                lvw_t = [hp.tile([128, E], f32r, name=f"lvw{kc}", tag=f"lvw{kc}")
                         for kc in range(4)]
                for kc in range(4):
                    nc.sync.dma_start(
                        lvw_t[kc], lvw_d[hof + kc * 128:hof + (kc + 1) * 128, :])
                    nc.vector.tensor_mul(lvw_t[kc], lvw_t[kc], MOD["A1"])
                for t in range(8):
                    py = psum_mm.tile([128, 512], f32, name="py", tag="pmm")
                    for kc in range(4):
                        nc.tensor.matmul(
                            py, oT[kc][:, t * 128:(t + 1) * 128], lvw_t[kc],
                            start=(kc == 0), stop=(kc == 3))
                    nc.vector.tensor_add(y2acc[t], y2acc[t], py)
                    if ln2_cb is not None:
                        ln2_cb(t)

            # Pipelined loop. PE stream per iteration:
            #   QK(h) -> sums(h-1) -> AV(h-1) -> V(h) -> scores(h) -> lv(h-1)
            # so exp/tree/recip of h-1 all hide under dense PE work.
            prev = None
            prev_oT = None
            for h in range(H):
                QT, KT = head_qk(h, pre=(PRE0 if h == 0 else None))
                if prev is not None:
                    Rrep = head_sums(prev[3])
                    prev_oT = head_av(prev[1], prev[2], Rrep)
                Vh = head_v(h)
                Eh, esum = head_scores(QT, KT)
                if prev is not None:
                    head_lv(prev[0], prev_oT)
                if h == 0:
                    emit_adaln_a1_seed()
                elif h == 2:
                    emit_adaln_ln2()
                elif h >= 4:
                    emit_f1w_quarter(h - 4)
                prev = (h, Vh, Eh, esum)
            Rrep = head_sums(prev[3])
            prev_oT = head_av(prev[1], prev[2], Rrep)
            head_lv(prev[0], prev_oT, ln2_cb=LN2_CB[0])
        hp_ctx.close()

        # ---- Phase 5 prefetch: FFN weights (hidden under residual/LN2) ----
        fp = ctx.enter_context(tc.tile_pool(name="fp", bufs=1))

        # keep the PE clock-gate open across the LN2/weight-DMA boundary
        for i in range(12):
            warm2 = psum_mm.tile([128, 512], f32, name="warm2", tag="pmm")
            nc.tensor.matmul(warm2[:, 0:128], ident, ident,
                             start=True, stop=True)

        # ---- Phase 5: FFN ----
        hT = [fp.tile([128, S], f32r, name=f"hT{hc}", tag=f"hT{hc}")
              for hc in range(16)]
        for sh in range(2):
            for hc in range(16):
                pf = psum_mm.tile([128, 512], f32, name="pf", tag="pmm")
                for kc in range(4):
                    nc.tensor.matmul(
                        pf, f1w_t[kc][:, hc * 128:(hc + 1) * 128],
                        zT[kc][:, sh * 512:(sh + 1) * 512],
                        start=(kc == 0), stop=(kc == 3))
                nc.scalar.activation(
                    hT[hc][:, sh * 512:(sh + 1) * 512], pf, AF.Relu,
                    bias=f1bc[:, hc:hc + 1], scale=1.0)
        f2w_t = [fp.tile([128, E], f32r, name=f"f2w{kc}", tag=f"f2w{kc}")
                 for kc in range(16)]
        for kc in range(16):
            nc.sync.dma_start(f2w_t[kc], f2w_d[kc * 128:(kc + 1) * 128, :])
        F2B_rep = fp.tile([128, E], f32, tag="f2brep")
        nc.sync.dma_start(F2B_rep, f2b_d.broadcast_to([128, E]))
        for t in range(8):
            pz = psum_mm.tile([128, 512], f32, name="pz", tag="pmm")
            for kc in range(16):
                nc.tensor.matmul(
                    pz, hT[kc][:, t * 128:(t + 1) * 128], f2w_t[kc],
                    start=(kc == 0), stop=(kc == 15))
            q1 = work.tile([128, E], f32, name="q1", tag="wbig")
            nc.vector.tensor_add(q1, pz, F2B_rep)
            nc.vector.tensor_mul(q1, q1, MOD["A2"])
            ot = work.tile([128, E], f32, name="ot", tag="wbig")
            nc.vector.tensor_add(ot, q1, y2acc[t])
            nc.sync.dma_start(out_d[t * 128:(t + 1) * 128, :], ot)

    nc.compile()
    return nc


def _get_program():
    if "nc" not in _CACHE:
        _CACHE["nc"] = _build()
    return _CACHE["nc"]


def kernel(**inputs) -> np.ndarray:
    from concourse.bass_utils import run_bass_kernel_spmd

    ins = {k: np.asarray(v, dtype=np.float32) for k, v in inputs.items()}
    nc = _get_program()

    in_maps = []
    for b in range(B):
        m = {
            "x": ins["x"][b],                       # (S, E)
            "cond": ins["cond"][b].reshape(E, 1),   # (E, 1)
            "ln1g": ins["ln1g"].reshape(1, E), "ln1b": ins["ln1b"].reshape(1, E),
            "ln2g": ins["ln2g"].reshape(1, E), "ln2b": ins["ln2b"].reshape(1, E),
            "wq": ins["wq"], "wk": ins["wk"], "wv": ins["wv"],
            "bq": ins["bq"].reshape(1, HE),
            "bk_s": (ins["bk"] * S_QK).reshape(1, HE),
            "lvw": ins["lvw"],
            # o is softmax(scores) @ (V + bv) = softmax(scores) @ V + bv,
            # so bv folds into the lv output bias: lvb_eff = lvb + bv @ lvw.
            "lvb": (ins["lvb"] + ins["bv"] @ ins["lvw"]).reshape(1, E),
            "f1w": ins["f1w"], "f1b": ins["f1b"].reshape(1, FF),
            "f2w": ins["f2w"], "f2b": ins["f2b"].reshape(1, E),
            "ident": np.eye(128, dtype=np.float32),
        }
        for nm in ["g1", "be1", "a1", "g2", "be2", "a2"]:
            m[f"{nm}w"] = ins[f"{nm}w"]
            m[f"{nm}b"] = ins[f"{nm}b"].reshape(1, E)
        in_maps.append(m)

    res = run_bass_kernel_spmd(nc, in_maps, list(range(N_CORES)),
                               trace=TRACE, tmpdir=TRACE_DIR)
    _CACHE["last_result"] = res
    out = np.stack([res.results[b]["out"] for b in range(B)], axis=0)
    return out



# revision 25
# speedup vs baseline: 1.2376x; 1.1052x over previous
"""DiT block kernel for Trainium2, data-parallel over batch across 8 NeuronCores.

Problem: nn_DiTBlock — B=8, S=1024, E=512, H=8 (head_dim = E = 512).
Sharding: batch element b -> core b. Each core runs the full DiT block on its
(S, E) slice with replicated weights; no collectives.

Per-core dataflow (activations kept transposed so the contraction dim sits on
partitions; float32r matmuls = full-rate PE with ~1.5e-4 relative rounding):
  AdaLN matvecs (PE) -> LN1 stats (DVE bn_stats) -> PE-transpose with fused
  modulate (ACT per-partition scale/bias) -> y^T
  -> software-pipelined head loop: QK(h) emitted between scores(h-1) and
     AV(h-1) so exp/tree latency hides under PE work
  -> residual -> LN2 (same transpose-modulate trick) -> FFN (relu in ACT)
  -> out = y + (h @ f2w + f2b) * alpha2.
"""
import sys
import numpy as np

sys.path.insert(0, '/opt/trn_rl_repo')

B, S, E, H = 8, 1024, 512, 8
HE = H * E          # 4096
FF = 4 * E          # 2048
EPS = 1e-5
SCALE = 1.0 / 32.0  # 1/sqrt(S)
N_CORES = 8

# fp8 (e4m3) quantization scales for the scores/AV DoubleRow path.
# Ranges (empirical max over the reference inputs, 4x+ margin to the 240
# clip): |Q|,|K| <= ~3.1 -> x16 = 49; exp(scores/32) <= ~3.3 -> x16 = 53;
# |V| <= ~3.2 -> x32 = 100.  |o| <= max|V| (convex combination).
S_QK = 16.0
S_EXP = 16.0
S_V = 32.0
S_Y = 8.0            # |y| <= ~11 -> x8 = 88
S_W = 64.0           # |w| <= ~0.11 -> x64 = 7
CD = 1.0 / (S_Y * S_W)   # dequant of yT8 @ w8 psum
import math as _math
EXP_BIAS = _math.log(S_EXP)           # exp(x)*S_EXP = exp(x + ln S_EXP)
SCL_SCORES = SCALE / (S_QK * S_QK)    # dequant of QT8.KT8 psum into exp

TRACE = False       # set by test harness to capture an NTFF profile
TRACE_DIR = None

_CACHE = {}


def _build():
    from contextlib import ExitStack
    import concourse.bass as bass
    import concourse.tile as tile
    from concourse import bacc, mybir
    f32 = mybir.dt.float32
    f32r = mybir.dt.float32r
    bf16 = mybir.dt.bfloat16
    fp8 = mybir.dt.float8e4
    PM = mybir.MatmulPerfMode.DoubleRow
    AF = mybir.ActivationFunctionType
    ALU = mybir.AluOpType

    nc = bacc.Bacc("TRN2", target_bir_lowering=False, debug=False,
                   num_devices=N_CORES)

    # ---- DRAM parameters --------------------------------------------------
    x_d = nc.dram_tensor("x", [S, E], f32, kind="ExternalInput").ap()
    cond_d = nc.dram_tensor("cond", [E, 1], f32, kind="ExternalInput").ap()

    adaln_w = {}
    adaln_b = {}
    for nm in ["g1", "be1", "a1", "g2", "be2", "a2"]:
        adaln_w[nm] = nc.dram_tensor(f"{nm}w", [E, E], f32,
                                     kind="ExternalInput").ap()
        adaln_b[nm] = nc.dram_tensor(f"{nm}b", [1, E], f32,
                                     kind="ExternalInput").ap()
    ln1g_d = nc.dram_tensor("ln1g", [1, E], f32, kind="ExternalInput").ap()
    ln1b_d = nc.dram_tensor("ln1b", [1, E], f32, kind="ExternalInput").ap()
    ln2g_d = nc.dram_tensor("ln2g", [1, E], f32, kind="ExternalInput").ap()
    ln2b_d = nc.dram_tensor("ln2b", [1, E], f32, kind="ExternalInput").ap()
    # QKV weights host-quantized to fp8*S_W in DoubleRow pair layout:
    # [pair g, partition p, sub i, out] with in-row e = g*256 + i*128 + p.
    wq_d = nc.dram_tensor("wq8", [2, 128, 2, HE], fp8, kind="ExternalInput").ap()
    wk_d = nc.dram_tensor("wk8", [2, 128, 2, HE], fp8, kind="ExternalInput").ap()
    wv_d = nc.dram_tensor("wv8", [2, 128, 2, HE], fp8, kind="ExternalInput").ap()
    bq_d = nc.dram_tensor("bq_s", [1, HE], f32, kind="ExternalInput").ap()
    # bk pre-scaled by S_QK on host (folded into the KT8 quantize ACT);
    # bv is folded into lvb on host (o = softmax-avg of V is bias-affine).
    bk_d = nc.dram_tensor("bk_s", [1, HE], f32, kind="ExternalInput").ap()
    lvw_d = nc.dram_tensor("lvw", [HE, E], f32r, kind="ExternalInput").ap()
    lvb_d = nc.dram_tensor("lvb", [1, E], f32, kind="ExternalInput").ap()
    f1w_d = nc.dram_tensor("f1w", [E, FF], f32r, kind="ExternalInput").ap()
    f1b_d = nc.dram_tensor("f1b", [1, FF], f32, kind="ExternalInput").ap()
    f2w_d = nc.dram_tensor("f2w", [FF, E], f32r, kind="ExternalInput").ap()
    f2b_d = nc.dram_tensor("f2b", [1, E], f32, kind="ExternalInput").ap()
    ident_d = nc.dram_tensor("ident", [128, 128], f32r,
                             kind="ExternalInput").ap()
    out_d = nc.dram_tensor("out", [S, E], f32, kind="ExternalOutput").ap()

    with tile.TileContext(nc) as tc, ExitStack() as ctx:
        const = ctx.enter_context(tc.tile_pool(name="const", bufs=1))
        work = ctx.enter_context(tc.tile_pool(name="work", bufs=3))
        psum_mm = ctx.enter_context(
            tc.tile_pool(name="psum_mm", bufs=5, space="PSUM"))
        psum_tp = ctx.enter_context(
            tc.tile_pool(name="psum_tp", bufs=2, space="PSUM"))
        psum_row = ctx.enter_context(
            tc.tile_pool(name="psum_row", bufs=1, space="PSUM"))

        # ---- constants ----
        ident = const.tile([128, 128], f32r)
        nc.sync.dma_start(ident, ident_d)
        eps_t = const.tile([128, 1], f32)
        nc.vector.memset(eps_t, EPS)
        # stationary column for the softmax-denominator matvec; its value
        # S_V folds V's fp8 dequant into the reciprocal (oT = po * Rrep).
        ones_col = const.tile([128, 1], f32)
        nc.vector.memset(ones_col, S_V)
        expb_t = const.tile([128, 1], f32)
        nc.vector.memset(expb_t, EXP_BIAS)

        cond_cols = const.tile([128, 4], f32)
        nc.sync.dma_start(cond_cols, cond_d.rearrange("(c p) o -> p (c o)", p=128))

        adp = ctx.enter_context(tc.tile_pool(name="adp", bufs=2))

        def adaln_cols(nm):
            """(cond @ W + b) laid out as [128, 4] e-columns (PE matvec)."""
            pcol = psum_tp.tile([128, 4], f32, name=f"pcol_{nm}", tag="ptp")
            for kc in range(4):
                adw = adp.tile([128, E], f32, name=f"adw_{nm}_{kc}", tag="adw")
                nc.sync.dma_start(adw, adaln_w[nm][kc * 128:(kc + 1) * 128, :])
                for ec in range(4):
                    nc.tensor.matmul(
                        pcol[:, ec:ec + 1],
                        adw[:, ec * 128:(ec + 1) * 128],
                        cond_cols[:, kc:kc + 1],
                        start=(kc == 0 and ec == 0),
                        stop=(kc == 3 and ec == 3))
            bcol = adp.tile([128, 4], f32, name=f"bcol_{nm}", tag="bcol")
            nc.sync.dma_start(
                bcol, adaln_b[nm].rearrange("o (c p) -> (o p) c", p=128))
            mcol = const.tile([128, 4], f32, name=f"mcol_{nm}")
            nc.vector.tensor_add(mcol, pcol, bcol)
            return mcol

        def adaln_rep(nm):
            """(cond @ W + b) replicated to [128, E] (row matvec + bcast)."""
            prow = psum_row.tile([1, E], f32, name=f"prow_{nm}", tag="prow")
            for kc in range(4):
                adw = adp.tile([128, E], f32, name=f"adw_{nm}_{kc}", tag="adw")
                nc.sync.dma_start(adw, adaln_w[nm][kc * 128:(kc + 1) * 128, :])
                nc.tensor.matmul(prow, cond_cols[:, kc:kc + 1], adw,
                                 start=(kc == 0), stop=(kc == 3))
            brow = adp.tile([1, E], f32, name=f"brow_{nm}", tag="brow")
            nc.sync.dma_start(brow, adaln_b[nm])
            arow = adp.tile([1, E], f32, name=f"arow_{nm}", tag="arow")
            nc.vector.tensor_add(arow, prow, brow)
            arep = const.tile([128, E], f32, name=f"arep_{nm}")
            nc.gpsimd.partition_broadcast(arep, arow)
            return arep

        def scale_shift(li, gcol, bcol, lng_d, lnb_d, outscale=1.0):
            """sc = ln_g*(1+gamma)*outscale, bi = (ln_b*(1+gamma)+beta)*
            outscale, as [128,4] cols.  outscale folds fp8 quantization."""
            gp = const.tile([128, 4], f32, name=f"gp_{li}")
            nc.vector.tensor_scalar(gp, gcol, scalar1=1.0, scalar2=outscale,
                                    op0=ALU.add, op1=ALU.mult)
            lgc = adp.tile([128, 4], f32, name=f"lgc_{li}", tag="lgc")
            nc.sync.dma_start(lgc, lng_d.rearrange("o (c p) -> (o p) c", p=128))
            lbc = adp.tile([128, 4], f32, name=f"lbc_{li}", tag="lbc")
            nc.sync.dma_start(lbc, lnb_d.rearrange("o (c p) -> (o p) c", p=128))
            sc = const.tile([128, 4], f32, name=f"sc_{li}")
            nc.vector.tensor_mul(sc, lgc, gp)
            bi = const.tile([128, 4], f32, name=f"bi_{li}")
            nc.vector.tensor_mul(bi, lbc, gp)
            # bi currently ln_b*(1+g)*outscale; add beta*outscale:
            bco = const.tile([128, 4], f32, name=f"bco_{li}")
            nc.vector.tensor_scalar(bco, bcol, scalar1=outscale, scalar2=None,
                                    op0=ALU.mult)
            nc.vector.tensor_add(bi, bi, bco)
            return sc, bi

        # Only g1/be1 gate the LN1->y^T critical path; defer the rest.
        sc1, bi1 = scale_shift(0, adaln_cols("g1"), adaln_cols("be1"),
                               ln1g_d, ln1b_d, outscale=S_Y)

        # ---- persistent activation tiles ----
        # yT8: y^T * S_Y as fp8 DoubleRow pairs [128, 2, S], pair g holds
        # e-chunks {2g, 2g+1}.  Stationary for V, moving for Q/K.
        yT8 = [const.tile([128, 2, S], fp8, name=f"yT8_{g}") for g in range(2)]
        y2acc = [const.tile([128, E], f32, name=f"y2acc{t}") for t in range(8)]

        def ln_stats(x_t, tagp):
            st = work.tile([128, 6], f32, name=f"st_{tagp}", tag=f"st_{tagp}")
            nc.vector.bn_stats(st, x_t)
            mv = work.tile([128, 2], f32, name=f"mv_{tagp}", tag=f"mv_{tagp}")
            nc.vector.bn_aggr(mv, st)
            rs = work.tile([128, 1], f32, name=f"rs_{tagp}", tag=f"rs_{tagp}")
            nc.scalar.activation(rs, mv[:, 1:2], AF.Sqrt, bias=eps_t, scale=1.0)
            nc.vector.reciprocal(rs, rs)
            xn = work.tile([128, E], f32r, name=f"xn_{tagp}", tag="wxn")
            nc.vector.tensor_scalar(xn, x_t, scalar1=mv[:, 0:1], scalar2=rs,
                                    op0=ALU.subtract, op1=ALU.mult)
            return xn

        def layernorm_transpose(src_tiles, scol, bcol, tagp):
            """LN over free dim of [128, E] tiles; PE-transpose 128-blocks in
            pairs sharing one PSUM bank; one fused modulate+quantize ACT per
            pair writing fp8 into the yT8 pair layout."""
            for t0 in range(0, 8, 2):
                xns = []
                for t in (t0, t0 + 1):
                    if src_tiles is None:
                        x_t = work.tile([128, E], f32, name=f"xt_{tagp}",
                                        tag="wbig")
                        nc.sync.dma_start(x_t, x_d[t * 128:(t + 1) * 128, :])
                    else:
                        x_t = src_tiles[t]
                    xns.append(ln_stats(x_t, tagp))
                for ec in range(4):
                    tp = psum_tp.tile([128, 256], f32r, name=f"tp_{tagp}",
                                      tag="ptp")
                    nc.tensor.matmul(tp[:, 0:128],
                                     xns[0][:, ec * 128:(ec + 1) * 128], ident,
                                     is_transpose=True, start=True, stop=False)
                    nc.tensor.matmul(tp[:, 128:256],
                                     xns[1][:, ec * 128:(ec + 1) * 128], ident,
                                     is_transpose=True, start=False, stop=True)
                    nc.scalar.activation(
                        yT8[ec // 2][:, ec % 2, t0 * 128:(t0 + 2) * 128], tp,
                        AF.Identity,
                        bias=bcol[:, ec:ec + 1], scale=scol[:, ec:ec + 1])

        # f1w prefetched in quarter-chunks during heads 4-7 (small DMAs so
        # the sync queue never head-of-line blocks the per-head weights).
        ffp = ctx.enter_context(tc.tile_pool(name="ffp", bufs=1))
        f1w_t = [ffp.tile([128, FF], f32r, name=f"f1w{kc}", tag=f"f1w{kc}")
                 for kc in range(4)]
        f1bc = ffp.tile([128, 16], f32, tag="f1bc")

        def emit_f1w_quarter(q):
            for kc in range(4):
                nc.sync.dma_start(
                    f1w_t[kc][:, q * 512:(q + 1) * 512],
                    f1w_d[kc * 128:(kc + 1) * 128, q * 512:(q + 1) * 512])
            if q == 3:
                nc.sync.dma_start(
                    f1bc, f1b_d.rearrange("o (c p) -> (o p) c", p=128))

        hp_ctx = ExitStack()
        hp = hp_ctx.enter_context(tc.tile_pool(name="hp", bufs=1))

        def emit_qk_dma(h):
            hof = h * E
            wq_t = [hp.tile([128, 2, E], fp8, name=f"wq{g}", tag=f"wq{g}")
                    for g in range(2)]
            wk_t = [hp.tile([128, 2, E], fp8, name=f"wk{g}", tag=f"wk{g}")
                    for g in range(2)]
            for g in range(2):
                nc.gpsimd.dma_start(wq_t[g], wq_d[g, :, :, hof:hof + E])
                nc.gpsimd.dma_start(wk_t[g], wk_d[g, :, :, hof:hof + E])
            bqc = hp.tile([128, 4], f32, tag="bqc")
            nc.sync.dma_start(
                bqc, bq_d[0:1, hof:hof + E].rearrange("o (c p) -> (o p) c",
                                                      p=128))
            bkc = hp.tile([128, 4], f32, tag="bkc")
            nc.sync.dma_start(
                bkc, bk_d[0:1, hof:hof + E].rearrange("o (c p) -> (o p) c",
                                                      p=128))
            return wq_t, wk_t, bqc, bkc

        # ---- Phase 1: LN1 -> y^T (fp8) ----
        PRE0 = emit_qk_dma(0)
        layernorm_transpose(None, sc1, bi1, "ln1")

        # Deferred AdaLN is emitted from inside the head loop (see below) so
        # its weight DMAs stay off the phase-0/head-0 DMA critical path.
        MOD = {}

        def emit_adaln_a1_seed():
            MOD["A1"] = adaln_rep("a1")
            # y = x + (o@lvw + lvb)*a1 accumulated head by head with a1
            # folded into lvw; seed the accumulator with x + lvb*a1.
            LVBA = const.tile([128, E], f32)
            nc.sync.dma_start(LVBA, lvb_d.broadcast_to([128, E]))
            nc.vector.tensor_mul(LVBA, LVBA, MOD["A1"])
            for t in range(8):
                x_t3 = work.tile([128, E], f32, name="xt3", tag="wbig")
                nc.sync.dma_start(x_t3, x_d[t * 128:(t + 1) * 128, :])
                nc.vector.tensor_add(y2acc[t], x_t3, LVBA)

        def emit_adaln_ln2():
            MOD["sc2"], MOD["bi2"] = scale_shift(
                1, adaln_cols("g2"), adaln_cols("be2"), ln2g_d, ln2b_d)
            MOD["A2"] = adaln_rep("a2")

        # LN2 emitted per tile from inside the last head's lv loop, so its
        # DVE chain hides under the remaining lv matmuls.
        zT = [const.tile([128, S], f32r, name=f"zT{c}", tag=f"yT{c}")
              for c in range(4)]

        _ln2_pend = []

        def ln2_tile(t):
            _ln2_pend.append((t, ln_stats(y2acc[t], "ln2")))
            if len(_ln2_pend) < 2:
                return
            (ta, xa), (tb, xb) = _ln2_pend
            _ln2_pend.clear()
            assert tb == ta + 1
            for ec in range(4):
                tp2 = psum_tp.tile([128, 256], f32r, name="tp_ln2", tag="ptp")
                nc.tensor.matmul(tp2[:, 0:128], xa[:, ec * 128:(ec + 1) * 128],
                                 ident, is_transpose=True, start=True, stop=False)
                nc.tensor.matmul(tp2[:, 128:256], xb[:, ec * 128:(ec + 1) * 128],
                                 ident, is_transpose=True, start=False, stop=True)
                nc.scalar.activation(
                    zT[ec][:, ta * 128:(ta + 2) * 128], tp2, AF.Identity,
                    bias=MOD["bi2"][:, ec:ec + 1], scale=MOD["sc2"][:, ec:ec + 1])

        LN2_CB = [ln2_tile]

        # ---- Phase 2: attention heads (software-pipelined) ----
        if True:

            def head_qk(h, pre=None):
                """Compute Q^T, K^T for head h, quantized to fp8 in the
                DoubleRow pair layout [128, 2, S] (pair = mc//2)."""
                wq_t, wk_t, bqc, bkc = pre if pre is not None else emit_qk_dma(h)
                QT = [hp.tile([128, 2, S], fp8, name=f"QT8_{g}", tag=f"QT{g}")
                      for g in range(2)]
                KT = [hp.tile([128, 2, S], fp8, name=f"KT8_{g}", tag=f"KT{g}")
                      for g in range(2)]
                for mc in range(4):
                    g, i = mc // 2, mc % 2
                    for sh in range(2):
                        pq = psum_mm.tile([128, 512], f32, name="pq", tag="pmm")
                        for wg in range(2):
                            nc.tensor.matmul(
                                pq, wq_t[wg][:, :, mc * 128:(mc + 1) * 128],
                                yT8[wg][:, :, sh * 512:(sh + 1) * 512],
                                start=(wg == 0), stop=(wg == 1), perf_mode=PM)
                        # bqc host-prescaled by S_QK: pq*CD*S_QK + bq*S_QK
                        nc.vector.tensor_scalar(
                            QT[g][:, i, sh * 512:(sh + 1) * 512], pq,
                            scalar1=CD * S_QK, scalar2=bqc[:, mc:mc + 1],
                            op0=ALU.mult, op1=ALU.add)
                        pk = psum_mm.tile([128, 512], f32, name="pk", tag="pmm")
                        for wg in range(2):
                            nc.tensor.matmul(
                                pk, wk_t[wg][:, :, mc * 128:(mc + 1) * 128],
                                yT8[wg][:, :, sh * 512:(sh + 1) * 512],
                                start=(wg == 0), stop=(wg == 1), perf_mode=PM)
                        nc.scalar.activation(
                            KT[g][:, i, sh * 512:(sh + 1) * 512], pk,
                            AF.Identity, bias=bkc[:, mc:mc + 1],
                            scale=CD * S_QK)
                return QT, KT

            def head_v(h):
                """Load wv slice, compute V*S_V as fp8 pairs [128, 2, E]
                (pair = tc//2).  bv is folded into lvb on the host."""
                hof = h * E
                wv_t = [hp.tile([128, 2, E], fp8, name=f"wv{g}", tag=f"wv{g}")
                        for g in range(2)]
                for g in range(2):
                    nc.sync.dma_start(wv_t[g], wv_d[g, :, :, hof:hof + E])
                Vh = [hp.tile([128, 2, E], fp8, name=f"V8_{g}", tag=f"V{g}")
                      for g in range(4)]
                for tc_ in range(8):
                    pv = psum_mm.tile([128, 512], f32, name="pv", tag="pmm")
                    for wg in range(2):
                        nc.tensor.matmul(
                            pv, yT8[wg][:, :, tc_ * 128:(tc_ + 1) * 128],
                            wv_t[wg],
                            start=(wg == 0), stop=(wg == 1), perf_mode=PM)
                    nc.scalar.activation(
                        Vh[tc_ // 2][:, tc_ % 2, :], pv, AF.Identity,
                        scale=CD * S_V)
                return Vh

            def head_scores(QT, KT):
                """scores^T via fp8 DoubleRow + exp*S_EXP into fp8 pairs
                [128, 2, S] (pair = tc//2); DVE tree for denominators."""
                Eh = [hp.tile([128, 2, S], fp8, name=f"E8_{g}", tag=f"E{g}")
                      for g in range(4)]
                esum = hp.tile([128, S], f32, tag="esum")
                for tc_ in range(8):
                    for sh in range(2):
                        ps = psum_mm.tile([128, 512], f32, name="ps", tag="pmm")
                        for g in range(2):
                            nc.tensor.matmul(
                                ps, KT[g][:, :, tc_ * 128:(tc_ + 1) * 128],
                                QT[g][:, :, sh * 512:(sh + 1) * 512],
                                start=(g == 0), stop=(g == 1), perf_mode=PM)
                        nc.scalar.activation(
                            Eh[tc_ // 2][:, tc_ % 2, sh * 512:(sh + 1) * 512],
                            ps, AF.Exp, scale=SCL_SCORES, bias=expb_t)
                    if tc_ == 1:
                        nc.vector.tensor_add(
                            esum, Eh[0][:, 0, :], Eh[0][:, 1, :])
                    elif tc_ > 1:
                        nc.vector.tensor_add(
                            esum, esum, Eh[tc_ // 2][:, tc_ % 2, :])
                return Eh, esum

            def head_sums(esum):
                """Softmax denominators -> replicated reciprocal rows.
                Copy PSUM rows out fast (frees the bank), broadcast, then
                take the reciprocal across all 128 lanes."""
                srow = hp.tile([1, S], f32, tag="srow")
                for sh in range(2):
                    psr = psum_row.tile([1, 512], f32, name="psr", tag="prow")
                    nc.tensor.matmul(psr, ones_col,
                                     esum[:, sh * 512:(sh + 1) * 512],
                                     start=True, stop=True)
                    nc.vector.tensor_copy(srow[0:1, sh * 512:(sh + 1) * 512], psr)
                Srep = hp.tile([128, S], f32, tag="esum")
                nc.gpsimd.partition_broadcast(Srep, srow)
                Rrep = hp.tile([128, S], f32, tag="rrep")
                nc.vector.reciprocal_approx_fast(Rrep, Srep)
                return Rrep

            def head_av(Vh, Eh, Rrep):
                """AV via fp8 DoubleRow; 1/sum (with fp8 dequants folded via
                the ones_col=S_V trick) applied on the PSUM->SBUF move."""
                oT = [hp.tile([128, S], f32r, name=f"oT{ec}", tag=f"oT{ec}")
                      for ec in range(4)]
                for ec in range(4):
                    for sh in range(2):
                        po = psum_mm.tile([128, 512], f32, name="po", tag="pmm")
                        for g in range(4):
                            nc.tensor.matmul(
                                po, Vh[g][:, :, ec * 128:(ec + 1) * 128],
                                Eh[g][:, :, sh * 512:(sh + 1) * 512],
                                start=(g == 0), stop=(g == 3), perf_mode=PM)
                        nc.vector.tensor_mul(
                            oT[ec][:, sh * 512:(sh + 1) * 512], po,
                            Rrep[:, sh * 512:(sh + 1) * 512])
                return oT

            def head_lv(h, oT, ln2_cb=None):
                """lv partial accumulate (alpha1 pre-folded into lvw)."""
                hof = h * E
                lvw_t = [hp.tile([128, E], f32r, name=f"lvw{kc}", tag=f"lvw{kc}")
                         for kc in range(4)]
                for kc in range(4):
                    nc.sync.dma_start(
                        lvw_t[kc], lvw_d[hof + kc * 128:hof + (kc + 1) * 128, :])
                    nc.vector.tensor_mul(lvw_t[kc], lvw_t[kc], MOD["A1"])
                for t in range(8):
                    py = psum_mm.tile([128, 512], f32, name="py", tag="pmm")
                    for kc in range(4):
                        nc.tensor.matmul(
                            py, oT[kc][:, t * 128:(t + 1) * 128], lvw_t[kc],
                            start=(kc == 0), stop=(kc == 3))
                    nc.vector.tensor_add(y2acc[t], y2acc[t], py)
                    if ln2_cb is not None:
                        ln2_cb(t)

            # Pipelined loop. PE stream per iteration:
            #   QK(h) -> sums(h-1) -> AV(h-1) -> V(h) -> scores(h) -> lv(h-1)
            # so exp/tree/recip of h-1 all hide under dense PE work.
            prev = None
            prev_oT = None
            for h in range(H):
                QT, KT = head_qk(h, pre=(PRE0 if h == 0 else None))
                if prev is not None:
                    Rrep = head_sums(prev[3])
                    prev_oT = head_av(prev[1], prev[2], Rrep)
                Vh = head_v(h)
                Eh, esum = head_scores(QT, KT)
                if prev is not None:
                    head_lv(prev[0], prev_oT)
                if h == 0:
                    emit_adaln_a1_seed()
                elif h == 2:
                    emit_adaln_ln2()
                elif h >= 4:
                    emit_f1w_quarter(h - 4)
                prev = (h, Vh, Eh, esum)
            Rrep = head_sums(prev[3])
            prev_oT = head_av(prev[1], prev[2], Rrep)
            head_lv(prev[0], prev_oT, ln2_cb=LN2_CB[0])
        hp_ctx.close()

        # ---- Phase 5 prefetch: FFN weights (hidden under residual/LN2) ----
        fp = ctx.enter_context(tc.tile_pool(name="fp", bufs=1))

        # keep the PE clock-gate open across the LN2/weight-DMA boundary
        for i in range(12):
            warm2 = psum_mm.tile([128, 512], f32, name="warm2", tag="pmm")
            nc.tensor.matmul(warm2[:, 0:128], ident, ident,
                             start=True, stop=True)

        # ---- Phase 5: FFN ----
        hT = [fp.tile([128, S], f32r, name=f"hT{hc}", tag=f"hT{hc}")
              for hc in range(16)]
        for sh in range(2):
            for hc in range(16):
                pf = psum_mm.tile([128, 512], f32, name="pf", tag="pmm")
                for kc in range(4):
                    nc.tensor.matmul(
                        pf, f1w_t[kc][:, hc * 128:(hc + 1) * 128],
                        zT[kc][:, sh * 512:(sh + 1) * 512],
                        start=(kc == 0), stop=(kc == 3))
                nc.scalar.activation(
                    hT[hc][:, sh * 512:(sh + 1) * 512], pf, AF.Relu,
                    bias=f1bc[:, hc:hc + 1], scale=1.0)
        f2w_t = [fp.tile([128, E], f32r, name=f"f2w{kc}", tag=f"f2w{kc}")
                 for kc in range(16)]
        for kc in range(16):
            nc.sync.dma_start(f2w_t[kc], f2w_d[kc * 128:(kc + 1) * 128, :])
        F2B_rep = fp.tile([128, E], f32, tag="f2brep")
        nc.sync.dma_start(F2B_rep, f2b_d.broadcast_to([128, E]))
        for t in range(8):
            pz = psum_mm.tile([128, 512], f32, name="pz", tag="pmm")
            for kc in range(16):
                nc.tensor.matmul(
                    pz, hT[kc][:, t * 128:(t + 1) * 128], f2w_t[kc],
                    start=(kc == 0), stop=(kc == 15))
            q1 = work.tile([128, E], f32, name="q1", tag="wbig")
            nc.vector.tensor_add(q1, pz, F2B_rep)
            nc.vector.tensor_mul(q1, q1, MOD["A2"])
            ot = work.tile([128, E], f32, name="ot", tag="wbig")
            nc.vector.tensor_add(ot, q1, y2acc[t])
            nc.sync.dma_start(out_d[t * 128:(t + 1) * 128, :], ot)

    nc.compile()
    return nc


def _get_program():
    if "nc" not in _CACHE:
        _CACHE["nc"] = _build()
    return _CACHE["nc"]


def _q8w(w):
    """Quantize [E, N] weight to fp8*S_W in the DoubleRow pair layout
    [2, 128, 2, N] with in-row e = g*256 + i*128 + p."""
    import ml_dtypes
    ws = np.clip(w * S_W, -240.0, 240.0)
    return np.ascontiguousarray(
        ws.reshape(2, 2, 128, -1).transpose(0, 2, 1, 3)
    ).astype(ml_dtypes.float8_e4m3)


def kernel(**inputs) -> np.ndarray:
    from concourse.bass_utils import run_bass_kernel_spmd

    ins = {k: np.asarray(v, dtype=np.float32) for k, v in inputs.items()}
    nc = _get_program()
    wq8 = _q8w(ins["wq"])
    wk8 = _q8w(ins["wk"])
    wv8 = _q8w(ins["wv"])

    in_maps = []
    for b in range(B):
        m = {
            "x": ins["x"][b],                       # (S, E)
            "cond": ins["cond"][b].reshape(E, 1),   # (E, 1)
            "ln1g": ins["ln1g"].reshape(1, E), "ln1b": ins["ln1b"].reshape(1, E),
            "ln2g": ins["ln2g"].reshape(1, E), "ln2b": ins["ln2b"].reshape(1, E),
            "wq8": wq8, "wk8": wk8, "wv8": wv8,
            "bq_s": (ins["bq"] * S_QK).reshape(1, HE),
            "bk_s": (ins["bk"] * S_QK).reshape(1, HE),
            "lvw": ins["lvw"],
            # o is softmax(scores) @ (V + bv) = softmax(scores) @ V + bv,
            # so bv folds into the lv output bias: lvb_eff = lvb + bv @ lvw.
            "lvb": (ins["lvb"] + ins["bv"] @ ins["lvw"]).reshape(1, E),
            "f1w": ins["f1w"], "f1b": ins["f1b"].reshape(1, FF),
            "f2w": ins["f2w"], "f2b": ins["f2b"].reshape(1, E),
            "ident": np.eye(128, dtype=np.float32),
        }
        for nm in ["g1", "be1", "a1", "g2", "be2", "a2"]:
            m[f"{nm}w"] = ins[f"{nm}w"]
            m[f"{nm}b"] = ins[f"{nm}b"].reshape(1, E)
        in_maps.append(m)

    res = run_bass_kernel_spmd(nc, in_maps, list(range(N_CORES)),
                               trace=TRACE, tmpdir=TRACE_DIR)
    _CACHE["last_result"] = res
    out = np.stack([res.results[b]["out"] for b in range(B)], axis=0)
    return out

